# revision 27
# baseline (speedup 1.0000x reference)
"""GAT (2-layer, 4-head, N=4096) Bass kernel for 8 Trainium2 NeuronCores.

Sharding: destination-node rows are split across the 8 cores (512 rows each).
x / weights are replicated; each core receives its own column-block of adj^T.

Per-core layout ("layout B"): attention logits are built TRANSPOSED, as
e^T[j, i] tiles of [128 source nodes (partitions), P local dest rows (free)].
 - e^T = (f1_bcast + f2_scalar) + adjneg  in ONE fused DVE scalar_tensor_tensor
   (adjneg = 0 for edges, -300 for non-edges; exp flushes masked entries to
   ~1e-26 exactly like the reference's -9e15 masking flushes them to 0).
 - leaky-relu on ACT (Prelu table, same ACT table-set as Exp) or on DVE
   (fused (x*0.2) max x), selectable per-chunk for engine load balancing.
 - p = exp(lr - 10) -> bf16 (the -10 shift cancels in softmax, keeps row sums
   within DVE reciprocal range).
 - att @ h needs the contraction index j on partitions -> p^T is already in
   the right orientation: hp^T[f, i] = sum_j h[j, f] p^T[j, i] accumulated in
   PSUM over 32 j-chunks.  An extra ones-column in h yields the softmax row
   sums for free; normalization is a rank-1 broadcast matmul + one multiply.
Layer 2 needs h2 = hcat @ W_out for ALL nodes on every core: each core
computes its local rows and one AllGather of [512, 65] bf16 (h2 | f2) shares
them.  Everything else is row-local.
"""
import sys

sys.path.insert(0, "/opt/trn_rl_repo")

import numpy as np

import concourse.bass as bass
import concourse.mybir as mybir
import concourse.tile as tile
from concourse.alu_op_type import AluOpType

F32 = mybir.dt.float32
BF16 = mybir.dt.bfloat16
U8 = mybir.dt.uint8
AF = mybir.ActivationFunctionType
ALPHA = 0.2
EXP_SHIFT = -10.0  # softmax-invariant shift, keeps row sums < 2^42


def _split_drain_waits(nc, max_waits=1):
    """walrus CoreV3 CTRL lowering accepts only one sem wait per Drain;
    split the tile-generated end-of-kernel drain into a chain of drains."""
    n = 0
    for fn in nc.m.functions:
        for blk in fn.blocks:
            i = 0
            while i < len(blk.instructions):
                inst = blk.instructions[i]
                si = inst.sync_info
                if (isinstance(inst, mybir.InstDrain) and si is not None
                        and len(si.on_wait) > max_waits):
                    waits = list(si.on_wait)
                    si.on_wait = waits[:max_waits]
                    rest = waits[max_waits:]
                    chunks = [rest[j:j + max_waits]
                              for j in range(0, len(rest), max_waits)]
                    for ci, ch in enumerate(chunks):
                        pre = mybir.InstDrain(
                            name=f"{inst.name}-ws{n}-{ci}",
                            engine=inst.engine, ins=[], outs=[],
                            sync_info=mybir.SyncInfo(on_wait=ch, on_update=[]))
                        nc.register_instruction(pre)
                        blk.instructions.insert(i, pre)
                        i += 1
                    n += 1
                i += 1
    return n


def build_gat(N=4096, F=64, H=4, FP=64, NCLS=64, NCORES=8, use_prelu=True,
              dve_lrelu_every=3):
    """SPMD GAT graph, v3: per-chunk tiles for precise Tile dependencies,
    e-gen = TS-ptr(f1+f2) + broadcast-TT(+adjneg) on DVE (2x modes),
    leaky-relu/exp fused across 4 slabs on ACT, split AllGather overlapped
    with layer-2 compute."""
    P = N // NCORES
    C = N // 128
    CL = P // 128
    HF = H * FP
    KH = HF // 128
    NPAIR = C // 2
    assert P % 128 == 0 and HF % 128 == 0 and P <= 512 and C % 4 == 0

    nc = bass.Bass()
    xT_d = nc.declare_dram_parameter("xTb", (F, N), BF16, isOutput=False)
    xTl_d = nc.declare_dram_parameter("xTloc", (F, P), F32, isOutput=False)
    adj_d = nc.declare_dram_parameter("adjTu8", (N, P), U8, isOutput=False)
    Wall_d = nc.declare_dram_parameter("Wall", (F, HF), F32, isOutput=False)
    WTall_d = nc.declare_dram_parameter("WTall", (FP, H * F), F32, isOutput=False)
    aTh_d = nc.declare_dram_parameter("aTh", (FP, 2 * H), F32, isOutput=False)
    Wo_d = nc.declare_dram_parameter("Wo", (HF, NCLS), F32, isOutput=False)
    WoT_d = nc.declare_dram_parameter("WoT", (NCLS, HF), F32, isOutput=False)
    ao_d = nc.declare_dram_parameter("ao", (NCLS, 2), F32, isOutput=False)
    out_d = nc.declare_dram_parameter("outT", (NCLS, P), F32, isOutput=True)

    # split the h2 gather into two pieces (local-node halves) so the second
    # half transfers while layer-2 consumes the first; degenerate to one
    # piece when the local block is a single chunk.
    split_cc = CL >= 2
    npiece = 2 if CL >= 2 else 1
    cc_ins = [nc.dram_tensor(f"cc_in{i}", (P // npiece, NCLS + 1), BF16)
              for i in range(npiece)]
    if split_cc:
        cc_outs = [nc.dram_tensor(f"cc_out{i}", (N // 2, NCLS + 1), BF16,
                                  addr_space="Shared") for i in range(2)]
    else:
        cc_outs = [nc.dram_tensor("cc_out", (N, NCLS + 1), BF16,
                                  addr_space="Shared")]

    with tile.TileContext(nc) as tc:
        with tc.tile_pool(name="const", bufs=1) as cp, \
             tc.tile_pool(name="stage", bufs=1) as sp, \
             tc.tile_pool(name="work", bufs=4) as wp, \
             tc.tile_pool(name="post", bufs=1) as pp, \
             tc.tile_pool(name="psacc", bufs=4, space="PSUM") as ps_acc, \
             tc.tile_pool(name="psf1b", bufs=1, space="PSUM") as ps_f1b, \
             tc.tile_pool(name="psr", bufs=1, space="PSUM") as ps_r, \
             tc.tile_pool(name="psmisc", bufs=2, space="PSUM") as ps_m:

            # ---------------- staging / constants ----------------
            xTl_f = sp.tile([F, P], F32, tag="xTlf")

            WTall_f = sp.tile([FP, H * F], F32, tag="WTallf")
            aTh_f = sp.tile([FP, 2 * H], F32, tag="aThf")
            Wo_f = sp.tile([128, KH, NCLS], F32, tag="Wof")
            WoT_f = sp.tile([NCLS, HF], F32, tag="WoTf")
            ao_f = sp.tile([NCLS, 2], F32, tag="aof")
            Wall_f = sp.tile([F, HF], F32, tag="Wallf")

            xT_b = cp.tile([F, N], BF16, tag="xTb")
            xTl_b = cp.tile([F, P], BF16, tag="xTlb")
            WallE = cp.tile([F, HF + 2 * H], BF16, tag="WallE")
            Waco_f = cp.tile([F, 2 * H], F32, tag="Wacof")
            adjn_p = [cp.tile([128, 2, P], BF16, tag=f"adjn_{j}", name=f"adjn_{j}")
                      for j in range(NPAIR)]
            hsb_t = [cp.tile([128, H, FP + 1], BF16, tag=f"hsb_{j}", name=f"hsb_{j}")
                     for j in range(C)]
            f12c_t = [cp.tile([128, 2 * H], F32, tag=f"f12c_{j}", name=f"f12c_{j}")
                      for j in range(C)]
            f12r = cp.tile([2 * H, P], F32, tag="f12r")
            f1rows = cp.tile([1, H, P], F32, tag="f1rows")
            hcatT = cp.tile([128, KH, P], BF16, tag="hcatT")
            h2p_t = [cp.tile([128, NCLS + 4], BF16, tag=f"h2p_{j}", name=f"h2p_{j}")
                     for j in range(C)]
            f2b_t = [cp.tile([128, 1], F32, tag=f"f2b_{j}", name=f"f2b_{j}")
                     for j in range(C)]
            f1r2 = cp.tile([1, P], F32, tag="f1r2")
            WoE = cp.tile([128, KH, NCLS + 1], BF16, tag="WoE")
            w1a_b = cp.tile([128, KH], BF16, tag="w1a")
            ones = cp.tile([1, 128], F32, tag="ones")
            ones_b = cp.tile([1, 128], BF16, tag="ones_b")
            neg300 = cp.tile([128, 1], F32, tag="neg300")
            f1rows_b = cp.tile([1, H, P], BF16, tag="f1rows_b")
            neg10 = cp.tile([128, 1], F32, tag="neg10")
            h2g = cp.tile([128, CL, NCLS + 1], BF16, tag="h2g")
            F1bh = cp.tile([128, H, P], BF16, tag="F1bh")
            F1b2 = cp.tile([128, P], BF16, tag="F1b2")

            # ---------------- input DMAs ----------------
            nc.sync.dma_start(out=xTl_f[:], in_=xTl_d[:])
            nc.sync.dma_start(out=WTall_f[:], in_=WTall_d[:])
            nc.sync.dma_start(out=aTh_f[:], in_=aTh_d[:])
            nc.sync.dma_start(out=Wall_f[:], in_=Wall_d[:])
            nc.sync.dma_start(out=xT_b[:], in_=xT_d[:])


            nc.vector.memset(ones[:], 1.0)
            nc.vector.memset(ones_b[:], 1.0)
            nc.vector.memset(neg300[:], -300.0)
            nc.vector.memset(neg10[:], EXP_SHIFT)

            # ---------------- prep ----------------
            nc.vector.tensor_copy(xTl_b[:], xTl_f[:])
            nc.gpsimd.tensor_copy(WallE[:, 0:HF], Wall_f[:])

            waco_ps = ps_m.tile([F, 2 * H], F32, tag="misc")
            for h in range(H):
                for k in range(2):
                    nc.tensor.matmul(
                        waco_ps[:, 2 * h + k:2 * h + k + 1],
                        WTall_f[:, F * h:F * (h + 1)],
                        aTh_f[:, 2 * h + k:2 * h + k + 1],
                        start=True, stop=True)
            nc.vector.tensor_copy(Waco_f[:], waco_ps[:])
            nc.scalar.copy(out=WallE[:, HF:HF + 2 * H], in_=waco_ps[:])

            def prep_chunk(jc):
                """adj DMA + mask encode + h/f12 matmul + copies for chunk jc;
                emitted pipelined with the layer-1 loop."""
                au8 = wp.tile([128, P], U8, tag="adju8", name=f"adju8_{jc}")
                nc.sync.dma_start(out=au8[:],
                                  in_=adj_d[128 * jc:128 * (jc + 1), :])
                if jc % 4 != 3:
                    nc.vector.tensor_scalar(out=adjn_p[jc // 2][:, jc % 2, :],
                                            in0=au8[:],
                                            scalar1=300.0, scalar2=-300.0,
                                            op0=AluOpType.mult, op1=AluOpType.add)
                else:
                    nc.scalar.activation(out=adjn_p[jc // 2][:, jc % 2, :],
                                         in_=au8[:], func=AF.Identity,
                                         scale=300.0, bias=neg300[:, 0:1])
                hp_ps = ps_m.tile([128, HF + 2 * H], F32, tag="misc")
                nc.tensor.matmul(hp_ps[:],
                                 xT_b[:, 128 * jc:128 * (jc + 1)],
                                 WallE[:], start=True, stop=True)
                nc.scalar.copy(out=hsb_t[jc][:, :, 0:FP],
                               in_=hp_ps[:, 0:HF].rearrange("p (h f) -> p h f", h=H))
                nc.vector.tensor_copy(f12c_t[jc][:], hp_ps[:, HF:HF + 2 * H])
                nc.vector.memset(hsb_t[jc][:, :, FP], 1.0)

            f12r_ps = ps_m.tile([2 * H, P], F32, tag="misc")
            nc.tensor.matmul(f12r_ps[:], Waco_f[:], xTl_f[:], start=True, stop=True)
            nc.vector.tensor_copy(f12r[:], f12r_ps[:])
            for h in range(H):
                nc.sync.dma_start(out=f1rows[0:1, h, :],
                                  in_=f12r[2 * h:2 * h + 1, :])
            nc.vector.tensor_copy(f1rows_b[:], f1rows[:])
            for h in range(H):
                f1b_ps = ps_f1b.tile([128, P], F32, tag="f1bps")
                nc.tensor.matmul(f1b_ps[:], ones_b[:], f1rows_b[0:1, h, :],
                                 start=True, stop=True)
                nc.scalar.copy(out=F1bh[:, h, :], in_=f1b_ps[:])

            def slab_tail(e4, G, use_dve, lhs_list, hp_list, ss_list):
                """leaky-relu + exp over a fused [128, G, P] slab, then the G
                PSUM-accumulating attention matmuls."""
                lr4 = wp.tile([128, G, P], BF16, tag="lr4")
                if use_dve:
                    nc.vector.scalar_tensor_tensor(
                        out=lr4[:], in0=e4[:], scalar=ALPHA, in1=e4[:],
                        op0=AluOpType.mult, op1=AluOpType.max)
                else:
                    nc.scalar.activation(out=lr4[:], in_=e4[:],
                                         func=AF.Prelu, alpha=ALPHA)
                p4 = wp.tile([128, G, P], BF16, tag="p4")
                nc.scalar.activation(out=p4[:], in_=lr4[:], func=AF.Exp,
                                     bias=neg10[:, 0:1])
                for g in range(G):
                    st, sp_ = ss_list[g]
                    nc.tensor.matmul(hp_list[g], lhs_list[g], p4[:, g, :],
                                     start=st, stop=sp_)

            def normalize(hp_acc, M, out_tile):
                rinv = pp.tile([1, P], F32, tag="rinv")
                nc.vector.reciprocal(rinv[:], hp_acc[M:M + 1, :])
                R_ps = ps_r.tile([128, P], F32, tag="Rps")
                nc.tensor.matmul(R_ps[0:M, :], ones[:, 0:M], rinv[:],
                                 start=True, stop=True)
                R_sb = pp.tile([M, P], F32, tag="Rsb")
                nc.vector.tensor_copy(R_sb[:], R_ps[0:M, :])
                nc.vector.tensor_tensor(out=out_tile[:], in0=hp_acc[0:M, :],
                                        in1=R_sb[:], op=AluOpType.mult)

            # ---------------- layer 1 ----------------
            hp_accs = [ps_acc.tile([FP + 1, P], F32, tag="hp", name=f"hp{i}")
                       for i in range(H)]
            PRE = 6
            for jc in range(PRE):
                prep_chunk(jc)
            for jc in range(C):
                if jc + PRE < C:
                    prep_chunk(jc + PRE)
                e0 = wp.tile([128, H, P], BF16, tag="e0")
                for h in range(H):
                    nc.vector.tensor_scalar_add(
                        out=e0[:, h, :], in0=F1bh[:, h, :],
                        scalar1=f12c_t[jc][:, 2 * h + 1:2 * h + 2])
                e4 = wp.tile([128, H, P], BF16, tag="e4")
                adjb = adjn_p[jc // 2][:, jc % 2, :].unsqueeze(1).to_broadcast(
                    (128, H, P))
                nc.vector.tensor_tensor(out=e4[:], in0=e0[:], in1=adjb,
                                        op=AluOpType.add)
                slab_tail(e4, H,
                          (not use_prelu) or (dve_lrelu_every and
                                              jc % dve_lrelu_every == 0),
                          [hsb_t[jc][:, h, :] for h in range(H)],
                          [hp_accs[h][:] for h in range(H)],
                          [(jc == 0, jc == C - 1)] * H)

            # ---- layer-2 weight prep (independent of layer-1 post) ----
            nc.sync.dma_start(out=WoT_f[:], in_=WoT_d[:])
            nc.sync.dma_start(out=ao_f[:], in_=ao_d[:])
            for k in range(KH):
                nc.sync.dma_start(out=Wo_f[:, k, :],
                                  in_=Wo_d[128 * k:128 * (k + 1), :])
            for k in range(KH):
                w12_ps = ps_m.tile([128, 2], F32, tag="misc")
                for j in range(2):
                    nc.tensor.matmul(w12_ps[:, j:j + 1],
                                     WoT_f[:, 128 * k:128 * (k + 1)],
                                     ao_f[:, j:j + 1],
                                     start=True, stop=True)
                nc.scalar.copy(out=WoE[:, k, 1:NCLS + 1], in_=Wo_f[:, k, :])
                nc.scalar.copy(out=WoE[:, k, 0:1], in_=w12_ps[:, 1:2])
                nc.vector.tensor_copy(w1a_b[:, k:k + 1], w12_ps[:, 0:1])

            # ---- layer-1 post, split into column pieces so each gather
            # piece can start as soon as its local columns are done ----
            halves = [list(range(0, CL // 2)), list(range(CL // 2, CL))] \
                if split_cc else [list(range(CL))]
            PW = 128 * len(halves[0])
            for ci, half in enumerate(halves):
                cs = slice(PW * ci, PW * (ci + 1))
                u4 = pp.tile([FP, H, PW], BF16, tag="u4", name=f"u4_{ci}")
                for h in range(H):
                    rinv = pp.tile([1, PW], F32, tag="rinv", name=f"ri{ci}_{h}")
                    nc.vector.reciprocal(rinv[:], hp_accs[h][FP:FP + 1, cs])
                    R_ps = ps_r.tile([128, PW], F32, tag="Rps", name=f"R{ci}_{h}")
                    nc.tensor.matmul(R_ps[0:FP, :], ones[:, 0:FP], rinv[:],
                                     start=True, stop=True)
                    R_sb = pp.tile([FP, PW], F32, tag="Rsb", name=f"Rs{ci}_{h}")
                    nc.scalar.copy(out=R_sb[:], in_=R_ps[0:FP, :])
                    nc.vector.tensor_tensor(out=u4[:, h, :],
                                            in0=hp_accs[h][0:FP, cs],
                                            in1=R_sb[:], op=AluOpType.mult)
                t2 = pp.tile([FP, H, PW], BF16, tag="t2", name=f"t2_{ci}")
                nc.scalar.activation(out=t2[:], in_=u4[:], func=AF.Relu,
                                     scale=-1.0)
                t3 = pp.tile([FP, H, PW], BF16, tag="t3", name=f"t3_{ci}")
                nc.scalar.activation(out=t3[:], in_=t2[:], func=AF.Exp,
                                     scale=-1.0)
                r1m = pp.tile([FP, H, PW], BF16, tag="r1m", name=f"r1m_{ci}")
                nc.vector.tensor_scalar(out=r1m[:], in0=u4[:], scalar1=0.0,
                                        scalar2=-1.0, op0=AluOpType.max,
                                        op1=AluOpType.add)
                for h in range(H):
                    nc.vector.tensor_tensor(
                        out=hcatT[FP * (h % 2):FP * (h % 2) + FP, h // 2, cs],
                        in0=t3[:, h, :], in1=r1m[:, h, :], op=AluOpType.add)
                # local h2 rows for this piece + its gather
                for lc in half:
                    h2_ps = ps_m.tile([128, NCLS + 1], F32, tag="misc")
                    for k in range(KH):
                        nc.tensor.matmul(h2_ps[:],
                                         hcatT[:, k, 128 * lc:128 * (lc + 1)],
                                         WoE[:, k, :], start=(k == 0),
                                         stop=(k == KH - 1))
                    nc.vector.tensor_copy(h2g[:, lc, :], h2_ps[:])
                    lo = half.index(lc)
                    nc.sync.dma_start(
                        out=cc_ins[ci][128 * lo:128 * (lo + 1), :],
                        in_=h2g[:, lc, :])
                nc.gpsimd.collective_compute(
                    "AllGather", AluOpType.bypass,
                    replica_groups=[list(range(NCORES))],
                    ins=[cc_ins[ci][:]], outs=[cc_outs[ci][:]])

            f1r2_ps = ps_m.tile([1, P], F32, tag="misc")
            for k in range(KH):
                nc.tensor.matmul(f1r2_ps[:], w1a_b[:, k:k + 1], hcatT[:, k, :],
                                 start=(k == 0), stop=(k == KH - 1))
            nc.vector.tensor_copy(f1r2[:], f1r2_ps[:])
            f1b2_ps = ps_f1b.tile([128, P], F32, tag="f1bps")
            nc.tensor.matmul(f1b2_ps[:], ones[:], f1r2[:], start=True, stop=True)
            nc.scalar.copy(out=F1b2[:], in_=f1b2_ps[:])

            # DMA the gathered pieces back per destination chunk
            halves2 = halves
            chunk_order = []
            for i, half in enumerate(halves2):
                nlocal = len(half)
                for r in range(NCORES):
                    for li, lc in enumerate(half):
                        jc = CL * r + lc
                        blk = 128 * (nlocal * r + li)
                        nc.sync.dma_start(
                            out=h2p_t[jc][:, 1:NCLS + 2],
                            in_=cc_outs[i][blk:blk + 128, :])
                        chunk_order.append(jc)

            # ---------------- layer 2, pair-fused, piece order ----------------
            hp2_acc = ps_acc.tile([NCLS + 1, P], F32, tag="hp", name="hp2")
            quads = [chunk_order[i:i + 4] for i in range(0, C, 4)]
            for qi, qd in enumerate(quads):
                e0 = wp.tile([128, 4, P], BF16, tag="e0")
                for g, jc in enumerate(qd):
                    nc.vector.tensor_copy(f2b_t[jc][:], h2p_t[jc][:, 1:2])
                    nc.vector.memset(h2p_t[jc][:, NCLS + 2:NCLS + 4], 1.0)
                    nc.vector.tensor_scalar_add(out=e0[:, g, :], in0=F1b2[:],
                                                scalar1=f2b_t[jc][:])
                e4 = wp.tile([128, 4, P], BF16, tag="e4")
                for half2 in range(2):
                    pr0 = qd[2 * half2]
                    assert qd[2 * half2 + 1] == pr0 + 1 and pr0 % 2 == 0
                    nc.vector.tensor_tensor(
                        out=e4[:, 2 * half2:2 * half2 + 2, :],
                        in0=e0[:, 2 * half2:2 * half2 + 2, :],
                        in1=adjn_p[pr0 // 2][:], op=AluOpType.add)
                slab_tail(e4, 4,
                          (not use_prelu) or (dve_lrelu_every and
                                              qi % dve_lrelu_every == 0),
                          [h2p_t[jc][:, 2:NCLS + 3] for jc in qd],
                          [hp2_acc[:]] * 4,
                          [(qi == 0 and g == 0,
                            qi == len(quads) - 1 and g == 3)
                           for g in range(4)])

            outT_sb = pp.tile([NCLS, P], F32, tag="outT")
            normalize(hp2_acc, NCLS, outT_sb)
            nc.sync.dma_start(out=out_d[:], in_=outT_sb[:])

    import bass_rust as _bass_rust
    _bass_rust.generate_event_semaphores(nc)
    nc.finalize()
    return nc


def make_in_maps(x, W_heads, a_heads, W_out, a_out, adj, ncores=8):
    """Pure layout transforms (transpose / slice / dtype) -> per-core inputs."""
    N, F = x.shape
    H = W_heads.shape[0]
    P = N // ncores
    import ml_dtypes
    xT = np.ascontiguousarray(x.T.astype(np.float32))
    xTb = np.ascontiguousarray(x.T.astype(ml_dtypes.bfloat16))
    adjT = adj.T.astype(np.uint8)
    Wall = np.ascontiguousarray(
        np.concatenate([W_heads[h] for h in range(H)], axis=1).astype(np.float32))
    WTall = np.ascontiguousarray(
        np.concatenate([W_heads[h].T for h in range(H)], axis=1).astype(np.float32))
    FPh = a_heads.shape[1] // 2
    aTh = np.ascontiguousarray(
        a_heads.reshape(H, 2, FPh).transpose(2, 0, 1).reshape(FPh, 2 * H)
        .astype(np.float32))
    Wo = np.ascontiguousarray(W_out.astype(np.float32))
    WoT = np.ascontiguousarray(W_out.T.astype(np.float32))
    ao = np.ascontiguousarray(a_out.astype(np.float32).reshape(2, -1).T)
    in_maps = []
    for c in range(ncores):
        in_maps.append({
            "xTb": xTb,
            "xTloc": np.ascontiguousarray(xT[:, c * P:(c + 1) * P]),
            "adjTu8": np.ascontiguousarray(adjT[:, c * P:(c + 1) * P]),
            "Wall": Wall, "WTall": WTall, "aTh": aTh,
            "Wo": Wo, "WoT": WoT, "ao": ao,
        })
    return in_maps


_CACHE = {}


def _run(x, W_heads, a_heads, W_out, a_out, adj, trace=False, **bkw):
    from concourse.bass_utils import run_bass_kernel_spmd

    N, F = x.shape
    H, _, FP = W_heads.shape
    NCLS = W_out.shape[1]
    NCORES = 8
    key = (N, F, H, FP, NCLS) + tuple(sorted(bkw.items()))
    if key not in _CACHE:
        _CACHE[key] = build_gat(N=N, F=F, H=H, FP=FP, NCLS=NCLS, NCORES=NCORES,
                                **bkw)
    nc = _CACHE[key]
    in_maps = make_in_maps(x, W_heads, a_heads, W_out, a_out, adj, NCORES)
    res = run_bass_kernel_spmd(nc, in_maps, core_ids=list(range(NCORES)),
                               trace=trace)
    out = np.concatenate([res.results[c]["outT"].T for c in range(NCORES)], axis=0)
    return out.astype(np.float32), res


def kernel(x, W_heads, a_heads, W_out, a_out, adj):
    out, _ = _run(np.asarray(x), np.asarray(W_heads), np.asarray(a_heads),
                  np.asarray(W_out), np.asarray(a_out), np.asarray(adj))
    return out


# revision 28
# speedup vs baseline: 1.0132x; 1.0132x over previous
"""GAT (2-layer, 4-head, N=4096) Bass kernel for 8 Trainium2 NeuronCores.

Sharding: destination-node rows are split across the 8 cores (512 rows each).
x / weights are replicated; each core receives its own column-block of adj^T.

Per-core layout ("layout B"): attention logits are built TRANSPOSED, as
e^T[j, i] tiles of [128 source nodes (partitions), P local dest rows (free)].
 - e^T = (f1_bcast + f2_scalar) + adjneg  in ONE fused DVE scalar_tensor_tensor
   (adjneg = 0 for edges, -300 for non-edges; exp flushes masked entries to
   ~1e-26 exactly like the reference's -9e15 masking flushes them to 0).
 - leaky-relu on ACT (Prelu table, same ACT table-set as Exp) or on DVE
   (fused (x*0.2) max x), selectable per-chunk for engine load balancing.
 - p = exp(lr - 10) -> bf16 (the -10 shift cancels in softmax, keeps row sums
   within DVE reciprocal range).
 - att @ h needs the contraction index j on partitions -> p^T is already in
   the right orientation: hp^T[f, i] = sum_j h[j, f] p^T[j, i] accumulated in
   PSUM over 32 j-chunks.  An extra ones-column in h yields the softmax row
   sums for free; normalization is a rank-1 broadcast matmul + one multiply.
Layer 2 needs h2 = hcat @ W_out for ALL nodes on every core: each core
computes its local rows and one AllGather of [512, 65] bf16 (h2 | f2) shares
them.  Everything else is row-local.
"""
import sys

sys.path.insert(0, "/opt/trn_rl_repo")

import numpy as np

import concourse.bass as bass
import concourse.mybir as mybir
import concourse.tile as tile
from concourse.alu_op_type import AluOpType

F32 = mybir.dt.float32
BF16 = mybir.dt.bfloat16
U8 = mybir.dt.uint8
AF = mybir.ActivationFunctionType
ALPHA = 0.2
EXP_SHIFT = -10.0  # softmax-invariant shift, keeps row sums < 2^42


def _split_drain_waits(nc, max_waits=1):
    """walrus CoreV3 CTRL lowering accepts only one sem wait per Drain;
    split the tile-generated end-of-kernel drain into a chain of drains."""
    n = 0
    for fn in nc.m.functions:
        for blk in fn.blocks:
            i = 0
            while i < len(blk.instructions):
                inst = blk.instructions[i]
                si = inst.sync_info
                if (isinstance(inst, mybir.InstDrain) and si is not None
                        and len(si.on_wait) > max_waits):
                    waits = list(si.on_wait)
                    si.on_wait = waits[:max_waits]
                    rest = waits[max_waits:]
                    chunks = [rest[j:j + max_waits]
                              for j in range(0, len(rest), max_waits)]
                    for ci, ch in enumerate(chunks):
                        pre = mybir.InstDrain(
                            name=f"{inst.name}-ws{n}-{ci}",
                            engine=inst.engine, ins=[], outs=[],
                            sync_info=mybir.SyncInfo(on_wait=ch, on_update=[]))
                        nc.register_instruction(pre)
                        blk.instructions.insert(i, pre)
                        i += 1
                    n += 1
                i += 1
    return n


def build_gat(N=4096, F=64, H=4, FP=64, NCLS=64, NCORES=8, use_prelu=True,
              dve_lrelu_every=3):
    """SPMD GAT graph, v3: per-chunk tiles for precise Tile dependencies,
    e-gen = TS-ptr(f1+f2) + broadcast-TT(+adjneg) on DVE (2x modes),
    leaky-relu/exp fused across 4 slabs on ACT, split AllGather overlapped
    with layer-2 compute."""
    P = N // NCORES
    C = N // 128
    CL = P // 128
    HF = H * FP
    KH = HF // 128
    NPAIR = C // 2
    assert P % 128 == 0 and HF % 128 == 0 and P <= 512 and C % 4 == 0

    nc = bass.Bass()
    xT_d = nc.declare_dram_parameter("xTb", (F, N), BF16, isOutput=False)
    xTl_d = nc.declare_dram_parameter("xTloc", (F, P), F32, isOutput=False)
    adj_d = nc.declare_dram_parameter("adjTu8", (N, P), U8, isOutput=False)
    Wall_d = nc.declare_dram_parameter("Wall", (F, HF), F32, isOutput=False)
    WTall_d = nc.declare_dram_parameter("WTall", (FP, H * F), F32, isOutput=False)
    aTh_d = nc.declare_dram_parameter("aTh", (FP, 2 * H), F32, isOutput=False)
    Wo_d = nc.declare_dram_parameter("Wo", (HF, NCLS), F32, isOutput=False)
    WoT_d = nc.declare_dram_parameter("WoT", (NCLS, HF), F32, isOutput=False)
    ao_d = nc.declare_dram_parameter("ao", (NCLS, 2), F32, isOutput=False)
    out_d = nc.declare_dram_parameter("outT", (NCLS, P), F32, isOutput=True)

    # split the h2 gather into two pieces (local-node halves) so the second
    # half transfers while layer-2 consumes the first; degenerate to one
    # piece when the local block is a single chunk.
    split_cc = CL >= 2
    npiece = 2 if CL >= 2 else 1
    cc_ins = [nc.dram_tensor(f"cc_in{i}", (P // npiece, NCLS + 1), BF16)
              for i in range(npiece)]
    if split_cc:
        cc_outs = [nc.dram_tensor(f"cc_out{i}", (N // 2, NCLS + 1), BF16,
                                  addr_space="Shared") for i in range(2)]
    else:
        cc_outs = [nc.dram_tensor("cc_out", (N, NCLS + 1), BF16,
                                  addr_space="Shared")]

    with tile.TileContext(nc) as tc:
        with tc.tile_pool(name="const", bufs=1) as cp, \
             tc.tile_pool(name="stage", bufs=1) as sp, \
             tc.tile_pool(name="work", bufs=4) as wp, \
             tc.tile_pool(name="post", bufs=1) as pp, \
             tc.tile_pool(name="psacc", bufs=4, space="PSUM") as ps_acc, \
             tc.tile_pool(name="psf1b", bufs=1, space="PSUM") as ps_f1b, \
             tc.tile_pool(name="psr", bufs=1, space="PSUM") as ps_r, \
             tc.tile_pool(name="psmisc", bufs=2, space="PSUM") as ps_m:

            # ---------------- staging / constants ----------------
            xTl_f = sp.tile([F, P], F32, tag="xTlf")

            WTall_f = sp.tile([FP, H * F], F32, tag="WTallf")
            aTh_f = sp.tile([FP, 2 * H], F32, tag="aThf")
            Wo_f = sp.tile([128, KH, NCLS], F32, tag="Wof")
            WoT_f = sp.tile([NCLS, HF], F32, tag="WoTf")
            ao_f = sp.tile([NCLS, 2], F32, tag="aof")
            Wall_f = sp.tile([F, HF], F32, tag="Wallf")

            xT_b = cp.tile([F, N], BF16, tag="xTb")
            xTl_b = cp.tile([F, P], BF16, tag="xTlb")
            WallE = cp.tile([F, HF + 2 * H], BF16, tag="WallE")
            Waco_f = cp.tile([F, 2 * H], F32, tag="Wacof")
            adjn_p = [cp.tile([128, 2, P], BF16, tag=f"adjn_{j}", name=f"adjn_{j}")
                      for j in range(NPAIR)]
            hsb_t = [cp.tile([128, H, FP + 1], BF16, tag=f"hsb_{j}", name=f"hsb_{j}")
                     for j in range(C)]
            f12c_t = [cp.tile([128, 2 * H], F32, tag=f"f12c_{j}", name=f"f12c_{j}")
                      for j in range(C)]
            f12r = cp.tile([2 * H, P], F32, tag="f12r")
            f1rows = cp.tile([1, H, P], F32, tag="f1rows")
            hcatT = cp.tile([128, KH, P], BF16, tag="hcatT")
            h2p_t = [cp.tile([128, NCLS + 4], BF16, tag=f"h2p_{j}", name=f"h2p_{j}")
                     for j in range(C)]
            f2b_t = [cp.tile([128, 1], F32, tag=f"f2b_{j}", name=f"f2b_{j}")
                     for j in range(C)]
            f1r2 = cp.tile([1, P], F32, tag="f1r2")
            WoE = cp.tile([128, KH, NCLS + 1], BF16, tag="WoE")
            w1a_b = cp.tile([128, KH], BF16, tag="w1a")
            ones = cp.tile([1, 128], F32, tag="ones")
            ones_b = cp.tile([1, 128], BF16, tag="ones_b")
            neg300 = cp.tile([128, 1], F32, tag="neg300")
            f1rows_b = cp.tile([1, H, P], BF16, tag="f1rows_b")
            neg10 = cp.tile([128, 1], F32, tag="neg10")
            h2g = cp.tile([128, CL, NCLS + 1], BF16, tag="h2g")
            F1bh = cp.tile([128, H, P], BF16, tag="F1bh")
            F1b2 = cp.tile([128, P], BF16, tag="F1b2")

            # ---------------- input DMAs ----------------
            nc.sync.dma_start(out=xTl_f[:], in_=xTl_d[:])
            nc.sync.dma_start(out=WTall_f[:], in_=WTall_d[:])
            nc.sync.dma_start(out=aTh_f[:], in_=aTh_d[:])
            nc.sync.dma_start(out=Wall_f[:], in_=Wall_d[:])
            nc.sync.dma_start(out=xT_b[:], in_=xT_d[:])


            nc.vector.memset(ones[:], 1.0)
            nc.vector.memset(ones_b[:], 1.0)
            nc.vector.memset(neg300[:], -300.0)
            nc.vector.memset(neg10[:], EXP_SHIFT)

            # ---------------- prep ----------------
            nc.vector.tensor_copy(xTl_b[:], xTl_f[:])
            nc.gpsimd.tensor_copy(WallE[:, 0:HF], Wall_f[:])

            waco_ps = ps_m.tile([F, 2 * H], F32, tag="misc")
            for h in range(H):
                for k in range(2):
                    nc.tensor.matmul(
                        waco_ps[:, 2 * h + k:2 * h + k + 1],
                        WTall_f[:, F * h:F * (h + 1)],
                        aTh_f[:, 2 * h + k:2 * h + k + 1],
                        start=True, stop=True)
            nc.vector.tensor_copy(Waco_f[:], waco_ps[:])
            nc.scalar.copy(out=WallE[:, HF:HF + 2 * H], in_=waco_ps[:])

            def prep_chunk(jc):
                """adj DMA + mask encode + h/f12 matmul + copies for chunk jc;
                emitted pipelined with the layer-1 loop."""
                au8 = wp.tile([128, P], U8, tag="adju8", name=f"adju8_{jc}")
                nc.sync.dma_start(out=au8[:],
                                  in_=adj_d[128 * jc:128 * (jc + 1), :])
                nc.vector.tensor_scalar(out=adjn_p[jc // 2][:, jc % 2, :],
                                        in0=au8[:],
                                        scalar1=300.0, scalar2=-300.0,
                                        op0=AluOpType.mult, op1=AluOpType.add)
                hp_ps = ps_m.tile([128, HF + 2 * H], F32, tag="misc")
                nc.tensor.matmul(hp_ps[:],
                                 xT_b[:, 128 * jc:128 * (jc + 1)],
                                 WallE[:], start=True, stop=True)
                nc.scalar.copy(out=hsb_t[jc][:, :, 0:FP],
                               in_=hp_ps[:, 0:HF].rearrange("p (h f) -> p h f", h=H))
                nc.vector.tensor_copy(f12c_t[jc][:], hp_ps[:, HF:HF + 2 * H])
                nc.vector.memset(hsb_t[jc][:, :, FP], 1.0)

            f12r_ps = ps_m.tile([2 * H, P], F32, tag="misc")
            nc.tensor.matmul(f12r_ps[:], Waco_f[:], xTl_f[:], start=True, stop=True)
            nc.vector.tensor_copy(f12r[:], f12r_ps[:])
            for h in range(H):
                nc.sync.dma_start(out=f1rows[0:1, h, :],
                                  in_=f12r[2 * h:2 * h + 1, :])
            nc.vector.tensor_copy(f1rows_b[:], f1rows[:])
            for h in range(H):
                f1b_ps = ps_f1b.tile([128, P], F32, tag="f1bps")
                nc.tensor.matmul(f1b_ps[:], ones_b[:], f1rows_b[0:1, h, :],
                                 start=True, stop=True)
                nc.scalar.copy(out=F1bh[:, h, :], in_=f1b_ps[:])

            def slab_tail(e4, G, use_dve, lhs_list, hp_list, ss_list):
                """leaky-relu + exp over a fused [128, G, P] slab, then the G
                PSUM-accumulating attention matmuls."""
                lr4 = wp.tile([128, G, P], BF16, tag="lr4")
                if use_dve:
                    nc.vector.scalar_tensor_tensor(
                        out=lr4[:], in0=e4[:], scalar=ALPHA, in1=e4[:],
                        op0=AluOpType.mult, op1=AluOpType.max)
                else:
                    nc.scalar.activation(out=lr4[:], in_=e4[:],
                                         func=AF.Prelu, alpha=ALPHA)
                p4 = wp.tile([128, G, P], BF16, tag="p4")
                nc.scalar.activation(out=p4[:], in_=lr4[:], func=AF.Exp,
                                     bias=neg10[:, 0:1])
                for g in range(G):
                    st, sp_ = ss_list[g]
                    nc.tensor.matmul(hp_list[g], lhs_list[g], p4[:, g, :],
                                     start=st, stop=sp_)

            def normalize(hp_acc, M, out_tile):
                rinv = pp.tile([1, P], F32, tag="rinv")
                nc.vector.reciprocal(rinv[:], hp_acc[M:M + 1, :])
                R_ps = ps_r.tile([128, P], F32, tag="Rps")
                nc.tensor.matmul(R_ps[0:M, :], ones[:, 0:M], rinv[:],
                                 start=True, stop=True)
                R_sb = pp.tile([M, P], F32, tag="Rsb")
                nc.vector.tensor_copy(R_sb[:], R_ps[0:M, :])
                nc.vector.tensor_tensor(out=out_tile[:], in0=hp_acc[0:M, :],
                                        in1=R_sb[:], op=AluOpType.mult)

            # ---------------- layer 1 ----------------
            hp_accs = [ps_acc.tile([FP + 1, P], F32, tag="hp", name=f"hp{i}")
                       for i in range(H)]
            PRE = 6
            for jc in range(PRE):
                prep_chunk(jc)
            for jc in range(C):
                if jc + PRE < C:
                    prep_chunk(jc + PRE)
                e0 = wp.tile([128, H, P], BF16, tag="e0")
                for h in range(H):
                    nc.vector.tensor_scalar_add(
                        out=e0[:, h, :], in0=F1bh[:, h, :],
                        scalar1=f12c_t[jc][:, 2 * h + 1:2 * h + 2])
                e4 = wp.tile([128, H, P], BF16, tag="e4")
                adjb = adjn_p[jc // 2][:, jc % 2, :].unsqueeze(1).to_broadcast(
                    (128, H, P))
                nc.vector.tensor_tensor(out=e4[:], in0=e0[:], in1=adjb,
                                        op=AluOpType.add)
                slab_tail(e4, H,
                          (not use_prelu) or (dve_lrelu_every and
                                              jc % dve_lrelu_every == 0 and
                                              jc < 27),
                          [hsb_t[jc][:, h, :] for h in range(H)],
                          [hp_accs[h][:] for h in range(H)],
                          [(jc == 0, jc == C - 1)] * H)

            # ---- layer-2 weight prep (independent of layer-1 post) ----
            nc.sync.dma_start(out=WoT_f[:], in_=WoT_d[:])
            nc.sync.dma_start(out=ao_f[:], in_=ao_d[:])
            for k in range(KH):
                nc.sync.dma_start(out=Wo_f[:, k, :],
                                  in_=Wo_d[128 * k:128 * (k + 1), :])
            for k in range(KH):
                w12_ps = ps_m.tile([128, 2], F32, tag="misc")
                for j in range(2):
                    nc.tensor.matmul(w12_ps[:, j:j + 1],
                                     WoT_f[:, 128 * k:128 * (k + 1)],
                                     ao_f[:, j:j + 1],
                                     start=True, stop=True)
                nc.scalar.copy(out=WoE[:, k, 1:NCLS + 1], in_=Wo_f[:, k, :])
                nc.scalar.copy(out=WoE[:, k, 0:1], in_=w12_ps[:, 1:2])
                nc.vector.tensor_copy(w1a_b[:, k:k + 1], w12_ps[:, 0:1])

            # ---- layer-1 post, split into column pieces so each gather
            # piece can start as soon as its local columns are done ----
            halves = [list(range(0, CL // 2)), list(range(CL // 2, CL))] \
                if split_cc else [list(range(CL))]
            PW = 128 * len(halves[0])
            for ci, half in enumerate(halves):
                cs = slice(PW * ci, PW * (ci + 1))
                u4 = pp.tile([FP, H, PW], BF16, tag="u4", name=f"u4_{ci}")
                for h in range(H):
                    rinv = pp.tile([1, PW], F32, tag="rinv", name=f"ri{ci}_{h}")
                    nc.vector.reciprocal(rinv[:], hp_accs[h][FP:FP + 1, cs])
                    R_ps = ps_r.tile([128, PW], F32, tag="Rps", name=f"R{ci}_{h}")
                    nc.tensor.matmul(R_ps[0:FP, :], ones[:, 0:FP], rinv[:],
                                     start=True, stop=True)
                    R_sb = pp.tile([FP, PW], F32, tag="Rsb", name=f"Rs{ci}_{h}")
                    nc.scalar.copy(out=R_sb[:], in_=R_ps[0:FP, :])
                    nc.vector.tensor_tensor(out=u4[:, h, :],
                                            in0=hp_accs[h][0:FP, cs],
                                            in1=R_sb[:], op=AluOpType.mult)
                t2 = pp.tile([FP, H, PW], BF16, tag="t2", name=f"t2_{ci}")
                nc.scalar.activation(out=t2[:], in_=u4[:], func=AF.Relu,
                                     scale=-1.0)
                t3 = pp.tile([FP, H, PW], BF16, tag="t3", name=f"t3_{ci}")
                nc.scalar.activation(out=t3[:], in_=t2[:], func=AF.Exp,
                                     scale=-1.0)
                r1m = pp.tile([FP, H, PW], BF16, tag="r1m", name=f"r1m_{ci}")
                nc.vector.tensor_scalar(out=r1m[:], in0=u4[:], scalar1=0.0,
                                        scalar2=-1.0, op0=AluOpType.max,
                                        op1=AluOpType.add)
                for h in range(H):
                    nc.vector.tensor_tensor(
                        out=hcatT[FP * (h % 2):FP * (h % 2) + FP, h // 2, cs],
                        in0=t3[:, h, :], in1=r1m[:, h, :], op=AluOpType.add)
                # local h2 rows for this piece + its gather
                for lc in half:
                    h2_ps = ps_m.tile([128, NCLS + 1], F32, tag="misc")
                    for k in range(KH):
                        nc.tensor.matmul(h2_ps[:],
                                         hcatT[:, k, 128 * lc:128 * (lc + 1)],
                                         WoE[:, k, :], start=(k == 0),
                                         stop=(k == KH - 1))
                    nc.vector.tensor_copy(h2g[:, lc, :], h2_ps[:])
                    lo = half.index(lc)
                    nc.sync.dma_start(
                        out=cc_ins[ci][128 * lo:128 * (lo + 1), :],
                        in_=h2g[:, lc, :])
                nc.gpsimd.collective_compute(
                    "AllGather", AluOpType.bypass,
                    replica_groups=[list(range(NCORES))],
                    ins=[cc_ins[ci][:]], outs=[cc_outs[ci][:]])

            f1r2_ps = ps_m.tile([1, P], F32, tag="misc")
            for k in range(KH):
                nc.tensor.matmul(f1r2_ps[:], w1a_b[:, k:k + 1], hcatT[:, k, :],
                                 start=(k == 0), stop=(k == KH - 1))
            nc.vector.tensor_copy(f1r2[:], f1r2_ps[:])
            f1b2_ps = ps_f1b.tile([128, P], F32, tag="f1bps")
            nc.tensor.matmul(f1b2_ps[:], ones[:], f1r2[:], start=True, stop=True)
            nc.scalar.copy(out=F1b2[:], in_=f1b2_ps[:])

            # DMA the gathered pieces back per destination chunk
            halves2 = halves
            chunk_order = []
            for i, half in enumerate(halves2):
                nlocal = len(half)
                for r in range(NCORES):
                    for li, lc in enumerate(half):
                        jc = CL * r + lc
                        blk = 128 * (nlocal * r + li)
                        nc.sync.dma_start(
                            out=h2p_t[jc][:, 1:NCLS + 2],
                            in_=cc_outs[i][blk:blk + 128, :])
                        chunk_order.append(jc)

            # ---------------- layer 2, pair-fused, piece order ----------------
            hp2_acc = ps_acc.tile([NCLS + 1, P], F32, tag="hp", name="hp2")
            quads = [chunk_order[i:i + 4] for i in range(0, C, 4)]
            for qi, qd in enumerate(quads):
                e0 = wp.tile([128, 4, P], BF16, tag="e0")
                for g, jc in enumerate(qd):
                    nc.vector.tensor_copy(f2b_t[jc][:], h2p_t[jc][:, 1:2])
                    nc.vector.memset(h2p_t[jc][:, NCLS + 2:NCLS + 4], 1.0)
                    nc.vector.tensor_scalar_add(out=e0[:, g, :], in0=F1b2[:],
                                                scalar1=f2b_t[jc][:])
                e4 = wp.tile([128, 4, P], BF16, tag="e4")
                for half2 in range(2):
                    pr0 = qd[2 * half2]
                    assert qd[2 * half2 + 1] == pr0 + 1 and pr0 % 2 == 0
                    nc.vector.tensor_tensor(
                        out=e4[:, 2 * half2:2 * half2 + 2, :],
                        in0=e0[:, 2 * half2:2 * half2 + 2, :],
                        in1=adjn_p[pr0 // 2][:], op=AluOpType.add)
                slab_tail(e4, 4,
                          (not use_prelu) or (dve_lrelu_every and
                                              qi % dve_lrelu_every == 0),
                          [h2p_t[jc][:, 2:NCLS + 3] for jc in qd],
                          [hp2_acc[:]] * 4,
                          [(qi == 0 and g == 0,
                            qi == len(quads) - 1 and g == 3)
                           for g in range(4)])

            outT_sb = pp.tile([NCLS, P], F32, tag="outT")
            for ti in range(2):
                ts_ = slice(P // 2 * ti, P // 2 * (ti + 1))
                rinv = pp.tile([1, P // 2], F32, tag="rinv", name=f"rto{ti}")
                nc.vector.reciprocal(rinv[:], hp2_acc[NCLS:NCLS + 1, ts_])
                R_ps = ps_r.tile([128, P // 2], F32, tag="Rps", name=f"Rto{ti}")
                nc.tensor.matmul(R_ps[0:NCLS, :], ones[:, 0:NCLS], rinv[:],
                                 start=True, stop=True)
                R_sb = pp.tile([NCLS, P // 2], F32, tag="Rsb", name=f"Rso{ti}")
                nc.scalar.copy(out=R_sb[:], in_=R_ps[0:NCLS, :])
                nc.vector.tensor_tensor(out=outT_sb[:, ts_],
                                        in0=hp2_acc[0:NCLS, ts_],
                                        in1=R_sb[:], op=AluOpType.mult)
                nc.sync.dma_start(out=out_d[:, ts_], in_=outT_sb[:, ts_])

    import bass_rust as _bass_rust
    _bass_rust.generate_event_semaphores(nc)
    nc.finalize()
    return nc


def make_in_maps(x, W_heads, a_heads, W_out, a_out, adj, ncores=8):
    """Pure layout transforms (transpose / slice / dtype) -> per-core inputs."""
    N, F = x.shape
    H = W_heads.shape[0]
    P = N // ncores
    import ml_dtypes
    xT = np.ascontiguousarray(x.T.astype(np.float32))
    xTb = np.ascontiguousarray(x.T.astype(ml_dtypes.bfloat16))
    adjT = adj.T.astype(np.uint8)
    Wall = np.ascontiguousarray(
        np.concatenate([W_heads[h] for h in range(H)], axis=1).astype(np.float32))
    WTall = np.ascontiguousarray(
        np.concatenate([W_heads[h].T for h in range(H)], axis=1).astype(np.float32))
    FPh = a_heads.shape[1] // 2
    aTh = np.ascontiguousarray(
        a_heads.reshape(H, 2, FPh).transpose(2, 0, 1).reshape(FPh, 2 * H)
        .astype(np.float32))
    Wo = np.ascontiguousarray(W_out.astype(np.float32))
    WoT = np.ascontiguousarray(W_out.T.astype(np.float32))
    ao = np.ascontiguousarray(a_out.astype(np.float32).reshape(2, -1).T)
    in_maps = []
    for c in range(ncores):
        in_maps.append({
            "xTb": xTb,
            "xTloc": np.ascontiguousarray(xT[:, c * P:(c + 1) * P]),
            "adjTu8": np.ascontiguousarray(adjT[:, c * P:(c + 1) * P]),
            "Wall": Wall, "WTall": WTall, "aTh": aTh,
            "Wo": Wo, "WoT": WoT, "ao": ao,
        })
    return in_maps


_CACHE = {}


def _run(x, W_heads, a_heads, W_out, a_out, adj, trace=False, **bkw):
    from concourse.bass_utils import run_bass_kernel_spmd

    N, F = x.shape
    H, _, FP = W_heads.shape
    NCLS = W_out.shape[1]
    NCORES = 8
    key = (N, F, H, FP, NCLS) + tuple(sorted(bkw.items()))
    if key not in _CACHE:
        _CACHE[key] = build_gat(N=N, F=F, H=H, FP=FP, NCLS=NCLS, NCORES=NCORES,
                                **bkw)
    nc = _CACHE[key]
    in_maps = make_in_maps(x, W_heads, a_heads, W_out, a_out, adj, NCORES)
    res = run_bass_kernel_spmd(nc, in_maps, core_ids=list(range(NCORES)),
                               trace=trace)
    out = np.concatenate([res.results[c]["outT"].T for c in range(NCORES)], axis=0)
    return out.astype(np.float32), res


def kernel(x, W_heads, a_heads, W_out, a_out, adj):
    out, _ = _run(np.asarray(x), np.asarray(W_heads), np.asarray(a_heads),
                  np.asarray(W_out), np.asarray(a_out), np.asarray(adj))
    return out
